# revision 43
# baseline (speedup 1.0000x reference)
"""Paged decode attention (nn_Attention_5626407157951) on 8 Trainium2 cores.

Tensor-parallel over heads: each core owns 4 of 32 heads. All operands are
bf16 single-term matmuls (no hi/lo error compensation) — rel err ~5e-3,
well inside the 2e-2 gate — which cuts both HBM bytes (81->53 MB/core)
and PE instruction count ~3x vs the compensated version.

Per core:
  qkv = hidden @ W_pack[:, own cols]       (bf16, fp32 acc)
  rotary(q, k) at pos=hist                 (DVE, fp32; host-built cos/sin)
  scores_T[s, (h,pair)] = K_cache^T q      (PE, K stationary bf16, q moving)
  softmax without max-subtraction; the last pair's validity mask is applied
  as the ACT bias operand of the exp; new token handled analytically:
      out = (sum_s exp(s)*v_s + e_new*v_new) / (sum_s exp(s) + e_new)
  out_partial = attn @ o_proj[:, own dims].T ; host sums the 8 partials.

DMA queue split: KV cache streams on the Sync HWDGE ring; weights
(W_pack, o_proj, hT) + outputs ride the Scalar HWDGE ring so the two
FIFO streams overlap and weight loads never queue behind KV prefetch.
KV pairs are packed compactly (only valid pairs shipped), K as
[d, pair, h, s] and V as [s, pair, h, d] so each per-request DMA is one
contiguous run per partition.
"""

import math
import os

import ml_dtypes
import numpy as np

import concourse.bass as bass
import concourse.mybir as mybir
import concourse.tile as tile
from concourse.bass_utils import run_bass_kernel_spmd
from concourse.vector_clock import ScopedClock

B = 32          # batch (decode requests)
H = 32          # total heads
HL = 4          # heads per core
D = 128         # head dim
HID = 4096
BS = 64         # cache block size
NBLK = 16       # blocks per request
NCORES = 8
KT = HID // 128         # 32 contraction tiles for qkv proj
PAIRS = NBLK // 2       # 8 block-pairs (128 tokens each) per request
ROPE_BASE = 10000.0
KV_PRE_ISSUE = 2        # kv groups queued before phase 1 (keep W_pack fast)
KV_DEPTH = 7            # kv group tiles in flight during attention
WP_CHUNK = 2            # kt tiles per W_pack DMA
GROUP_PAIRS = 6         # pack per-request kv DMAs to >= this many pairs
GROUP_CAP = 8           # ...but never beyond this many pairs per group
PIPE = 2                # attention software-pipeline depth (PV lags scores)

F32 = mybir.dt.float32
BF = mybir.dt.bfloat16
BF_NP = ml_dtypes.bfloat16
EXP_FN = mybir.ActivationFunctionType.Exp
MUL = mybir.AluOpType.mult
ADD = mybir.AluOpType.add
SUB = mybir.AluOpType.subtract

LAST_RESULTS = None  # test harness peeks at this for profiling info

# ---------------------------------------------------------------------------
# This walrus build accepts very few sync-waits per instruction; the Tile
# kernel-tail drain accumulates one wait per sem lane. Split the waits over
# several drain instructions (all before the barrier, so semantics hold).
_MAX_DRAIN_WAITS = 1


def _patched_drain_and_barrier(self, tick_clock, wait_clock):
    nc = self.nc
    drain_inst = nc.sync.drain()
    wait_clock.add_sem_waits(
        drain_inst.ins, ScopedClock({None: tick_clock.global_clock})
    )
    si = drain_inst.ins.sync_info
    if si is not None and si.on_wait and len(si.on_wait) > _MAX_DRAIN_WAITS:
        waits = list(si.on_wait)
        drain_inst.ins.sync_info = mybir.SyncInfo(
            on_wait=waits[:_MAX_DRAIN_WAITS], on_update=list(si.on_update or [])
        )
        rest = waits[_MAX_DRAIN_WAITS:]
        for i in range(0, len(rest), _MAX_DRAIN_WAITS):
            extra = nc.sync.drain()
            extra.ins.sync_info = mybir.SyncInfo(
                on_wait=rest[i : i + _MAX_DRAIN_WAITS], on_update=[]
            )
    nc.all_engine_barrier()
    popped = nc._tile_sem_poison_stack.pop()
    assert popped is self._sem_poison
    nc.clear_and_free_semaphores(list(self.sems.allocated().values()))
    nc.all_engine_barrier()


tile.TileContext._drain_and_barrier = _patched_drain_and_barrier


def _split_excess_waits(nc, limit=1):
    """Walrus rejects instructions carrying more than ~1 sync wait. Hoist the
    excess onto NoOps inserted just before, on the same engine queue (the
    queue blocks on them first, so semantics are identical)."""
    for fn in nc.m.functions:
        for bb in fn.blocks:
            out = []
            changed = False
            for inst in list(bb.instructions):
                si = getattr(inst, "sync_info", None)
                if si is not None and si.on_wait and len(si.on_wait) > limit:
                    waits = list(si.on_wait)
                    extra, keep = waits[:-limit], waits[-limit:]
                    for i in range(0, len(extra), limit):
                        nop = mybir.InstNoOp(
                            name=nc.get_next_instruction_name(),
                            ins=[], outs=[], engine=inst.engine,
                            sync_info=mybir.SyncInfo(
                                on_wait=extra[i : i + limit], on_update=[]
                            ),
                        )
                        nc.register_instruction(nop)
                        out.append(nop)
                    inst.sync_info = mybir.SyncInfo(
                        on_wait=keep, on_update=list(si.on_update or [])
                    )
                    changed = True
                out.append(inst)
            if changed:
                bb.instructions = out
# ---------------------------------------------------------------------------


def _build_nc(pairs):
    """Build the SPMD bass module. `pairs[b]` = number of 128-token cached
    pairs for request b (same on every core; head split is via input data)."""
    nc = bass.Bass()

    HD = HL * D  # 512 local attention dims
    SP = sum(pairs)
    offs = np.concatenate([[0], np.cumsum(pairs)]).astype(int)

    def param(name, shape, dt):
        return nc.declare_dram_parameter(name, list(shape), dt, isOutput=False)

    hT = param("hT", [128, KT, B], BF)
    wp = param("wp", [128, KT, 3 * HD], BF)
    wo = param("wo", [HL, 128, HID], BF)
    kvp = param("kv", [128, 2, max(SP, 1), HL, 128], BF)
    cs = param("cs", [B, 4 * HD], F32)
    ninvp = param("ninv", [1, HL * B], F32)
    identp = param("ident", [B, B], F32)
    out_part = nc.declare_dram_parameter("out_part", [B, HID], F32, isOutput=True)

    # Pack per-request kv loads into >= GROUP_PAIRS-pair DMA groups so small
    # requests don't pay per-transfer fixed latency each.
    groups = []          # list of lists of b
    cur, cur_p = [], 0
    for b in range(B):
        if pairs[b] == 0:
            continue
        if cur and cur_p + pairs[b] > GROUP_CAP:
            groups.append(cur)
            cur, cur_p = [], 0
        cur.append(b)
        cur_p += pairs[b]
        if cur_p >= GROUP_PAIRS:
            groups.append(cur)
            cur, cur_p = [], 0
    if cur:
        groups.append(cur)
    binfo = {}           # b -> (group idx, local pair offset)
    for gi, bs in enumerate(groups):
        lo = 0
        for b in bs:
            binfo[b] = (gi, lo)
            lo += pairs[b]
    gpairs = [sum(pairs[b] for b in bs) for bs in groups]
    goffs = [offs[bs[0]] for bs in groups]

    with tile.TileContext(nc) as tc:
        with (
            tc.tile_pool(name="const", bufs=1) as cpool,
            tc.tile_pool(name="work", bufs=1) as wpool,
            tc.tile_pool(name="wop", bufs=4) as wop,
            tc.tile_pool(name="kv", bufs=KV_DEPTH) as kvpool,
            tc.tile_pool(name="small", bufs=4) as smp,
        ):
            # ---- constants (sync ring, ahead of the KV stream) ----
            ident = cpool.tile([B, B], F32)
            nc.sync.dma_start(out=ident[:], in_=identp[:])
            ninv_sb = cpool.tile([1, HL * B], F32)
            nc.sync.dma_start(out=ninv_sb[:], in_=ninvp[:])
            cs_sb = cpool.tile([B, 4 * HD], F32)
            nc.sync.dma_start(out=cs_sb[:], in_=cs[:])
            # scalar ring leads with what the PE needs first
            hT_sb = cpool.tile([128, KT, B], BF)
            nc.scalar.dma_start(out=hT_sb[:], in_=hT[:])
            ones = cpool.tile([128, 1], BF)
            nc.vector.memset(ones[:], 1.0)
            onesf = cpool.tile([1, HL * B], F32)
            nc.vector.memset(onesf[:], 1.0)

            # KV loads: one DMA per group, K+V together, sync ring.
            # Only a couple go ahead of phase 1 so W_pack owns the wire.
            g_tiles = {}

            def load_g(gi):
                gp = gpairs[gi]
                o = goffs[gi]
                t = kvpool.tile([128, 2, gp, HL, 128], BF, tag="kv")
                nc.sync.dma_start(out=t[:], in_=kvp[:, :, o : o + gp, :, :])
                g_tiles[gi] = t

            for gi in range(min(KV_PRE_ISSUE, len(groups))):
                load_g(gi)

            # o_proj weights ride the sync ring right behind the first kv
            # groups: they stream during phase 1 and never block the scalar
            # queue (which phase 2's rope copies need).
            wo_tiles = {}
            for i in range(HL):
                t = wop.tile([128, HID], BF, tag="wot")
                nc.sync.dma_start(out=t[:], in_=wo[i])
                wo_tiles[i] = t

            # accumulators written per-b, read in the epilogue
            atsb = wpool.tile([128, HL * B], F32)   # cached attn, col h*32+b
            nc.vector.memset(atsb[:], 0.0)
            dnm = wpool.tile([1, HL * B], F32)      # cached denom, col h*32+b
            nc.vector.memset(dnm[:], 0.0)

            with tc.tile_pool(name="psA", bufs=1, space="PSUM") as psA:
                # PE warmup transpose so `ident` is observed by PE before the
                # real (fp32, single-wait-slot) transposes below.
                tp0 = psA.tile([B, B], F32, tag="tp0")
                nc.tensor.transpose(tp0[:], ident[:], ident[:])

                # ---- phase 1: qkv = hidden @ W_pack (bf16, chunked DMA) ----
                qkv_ps = psA.tile([B, 3 * HD], F32, tag="qkv")
                NCH = KT // WP_CHUNK
                with tc.tile_pool(name="wtiles", bufs=3) as wtp:
                    for ch in range(NCH):
                        k0 = ch * WP_CHUNK
                        wpt = wtp.tile([128, WP_CHUNK, 3 * HD], BF, tag="wpt")
                        nc.scalar.dma_start(
                            out=wpt[:], in_=wp[:, k0 : k0 + WP_CHUNK, :]
                        )
                        for j in range(WP_CHUNK):
                            for n in range(3):
                                nc.tensor.matmul(
                                    qkv_ps[:, n * HD : (n + 1) * HD],
                                    hT_sb[:, k0 + j, :],
                                    wpt[:, j, n * HD : (n + 1) * HD],
                                    start=(k0 + j == 0),
                                    stop=(k0 + j == KT - 1),
                                )
                qkv_sb = wpool.tile([B, 3 * HD], F32)
                nc.vector.tensor_copy(qkv_sb[:], qkv_ps[:])

                # ---- phase 2: rotary (fp32, DVE) + transposes ----
                def rope(src_off, cs_off):
                    src = qkv_sb[:, src_off : src_off + HD]
                    t1 = wpool.tile([B, HD], F32, tag="rope_t1")
                    nc.vector.tensor_tensor(
                        t1[:], src, cs_sb[:, cs_off : cs_off + HD], MUL
                    )
                    sh = wpool.tile([B, HD], F32, tag="rope_sh")
                    sh4 = sh[:].rearrange("b (h d) -> b h d", h=HL)
                    sr4 = qkv_sb[:, src_off : src_off + HD].rearrange(
                        "b (h d) -> b h d", h=HL
                    )
                    # rotate-half copies on the (idle) scalar engine so they
                    # overlap the DVE multiplies
                    nc.scalar.copy(sh4[:, :, 0:64], sr4[:, :, 64:128])
                    nc.scalar.copy(sh4[:, :, 64:128], sr4[:, :, 0:64])
                    nc.vector.tensor_tensor(
                        sh[:], sh[:], cs_sb[:, cs_off + HD : cs_off + 2 * HD], MUL
                    )
                    nc.vector.tensor_tensor(
                        qkv_sb[:, src_off : src_off + HD], t1[:], sh[:], ADD
                    )

                rope(0, 0)          # q (scale folded into tables)
                rope(HD, 2 * HD)    # k

                # PE transposes -> [128(d), (h,b)] fp32 tiles (pipelined)
                qT = wpool.tile([128, HL * B], F32)
                kT = wpool.tile([128, HL * B], F32)
                vT = wpool.tile([128, HL * B], F32)
                with tc.tile_pool(name="psT", bufs=2, space="PSUM") as psT:
                    for off, dst in ((0, qT), (HD, kT), (2 * HD, vT)):
                        for h in range(HL):
                            tp = psT.tile([128, B], F32, tag="tp")
                            inp = qkv_sb[:, off + h * D : off + (h + 1) * D]
                            nc.tensor.transpose(tp[:], inp, ident[:])
                            nc.vector.tensor_copy(
                                dst[:, h * B : (h + 1) * B], tp[:]
                            )

                qT_bf = wpool.tile([128, HL * B], BF)
                nc.vector.tensor_copy(qT_bf[:], qT[:])

                # new-token scores: e_new[(h,b)] = exp(q . k_new)
                prod = wpool.tile([128, HL * B], F32)
                nc.vector.tensor_tensor(prod[:], qT[:], kT[:], MUL)
                prod_bf = wpool.tile([128, HL * B], BF)
                nc.vector.tensor_copy(prod_bf[:], prod[:])
                sn_ps = psA.tile([1, HL * B], F32, tag="sn")
                nc.tensor.matmul(sn_ps[:], ones[:], prod_bf[:], start=True, stop=True)
                e_new = wpool.tile([1, HL * B], F32)
                nc.scalar.activation(e_new[:], sn_ps[:], EXP_FN)
                # broadcast e_new across partitions now (sbuf copy): the
                # split epilogue only needs the rec broadcast per half
                ebp = psA.tile([128, HL * B], F32, tag="ebp")
                nc.tensor.matmul(ebp[:], onesf[:], e_new[:], start=True, stop=True)
                ebs = wpool.tile([128, HL * B], F32)
                nc.vector.tensor_copy(ebs[:], ebp[:])

            dtot = wpool.tile([1, HL * B], F32)
            rec = wpool.tile([1, HL * B], F32)
            att_bf = wpool.tile([128, HL * B], BF)

            # ---- phase 3: group-batched paged attention ----
            # One scores->mask->exp->PV round trip per kv DMA group (not per
            # request), software-pipelined PIPE groups deep so the PE never
            # waits on an exp it just issued.
            with (
                tc.tile_pool(name="psS", bufs=3, space="PSUM") as psS,
                tc.tile_pool(name="psV", bufs=3, space="PSUM") as psV,
                tc.tile_pool(name="psB2", bufs=2, space="PSUM") as psB2,
            ):
                def do_pv(ent):
                    gi, bs, kvt, ph = ent
                    gp = gpairs[gi]
                    nb = len(bs)
                    b0 = bs[0]
                    atp = psV.tile([128, HL, nb], F32, tag="atp")
                    for h in range(HL):
                        for j, b in enumerate(bs):
                            lo = offs[b] - goffs[gi]
                            pb = pairs[b]
                            for p in range(pb):
                                nc.tensor.matmul(
                                    atp[:, h, j : j + 1],
                                    kvt[:, 1, lo + p, h, :],
                                    ph[:, h, lo + p : lo + p + 1],
                                    start=(p == 0), stop=(p == pb - 1),
                                )
                    nc.vector.tensor_copy(
                        atsb[:].rearrange("d (h b2) -> d h b2", h=HL)
                        [:, :, b0 : b0 + nb],
                        atp[:],
                    )
                    # denominators: column sums of probs, then per-b segments
                    dsp = psB2.tile([1, HL * gp], F32, tag="dsp")
                    nc.tensor.matmul(
                        dsp[:], ones[:], ph[:].rearrange("s h p -> s (h p)"),
                        start=True, stop=True,
                    )
                    dspv = dsp[:].rearrange("o (h p) -> o h p", h=HL)
                    for b in bs:
                        lo = offs[b] - goffs[gi]
                        nc.vector.reduce_sum(
                            dnm[:].rearrange("o (h b2) -> o h b2", h=HL)[:, :, b],
                            dspv[:, :, lo : lo + pairs[b]],
                            axis=mybir.AxisListType.X,
                        )

                stage = []
                for gi, bs in enumerate(groups):
                    if gi not in g_tiles:
                        load_g(gi)
                    for gn in range(gi + 1, min(gi + KV_DEPTH - 1, len(groups))):
                        if gn not in g_tiles:
                            load_g(gn)
                    kvt = g_tiles[gi]
                    gp = gpairs[gi]
                    o = goffs[gi]

                    # scores^T: [128(s), (h, pair)] for the whole group
                    scp = psS.tile([128, HL, gp], F32, tag="scp")
                    for h in range(HL):
                        for b in bs:
                            lo = offs[b] - o
                            qh = qT_bf[:, h * B + b : h * B + b + 1]
                            for p in range(pairs[b]):
                                nc.tensor.matmul(
                                    scp[:, h, lo + p : lo + p + 1],
                                    kvt[:, 0, lo + p, h, :],
                                    qh, start=True, stop=True,
                                )

                    # exp -> probs (bf16), straight from PSUM. No mask: the
                    # host zeroed K/V at invalid positions, so they land as
                    # exp(0)=1 times V=0; the denominator over-count is a
                    # host-known constant removed in the epilogue.
                    ph = smp.tile([128, HL, gp], BF, tag="ph")
                    nc.scalar.activation(ph[:], scp[:], EXP_FN)

                    stage.append((gi, bs, kvt, ph))
                    if len(stage) > PIPE:
                        do_pv(stage.pop(0))
                while stage:
                    do_pv(stage.pop(0))

            # ---- epilogue: add new token, normalize, project ----
            # dnm counts exp(0)=1 for each host-zeroed invalid slot; subtract
            # the known count, then add the new token's weight.
            nc.vector.tensor_tensor(dtot[:], dnm[:], ninv_sb[:], SUB)
            nc.vector.tensor_tensor(dtot[:], dtot[:], e_new[:], ADD)
            nc.vector.reciprocal(rec[:], dtot[:])
            att = wpool.tile([128, HL * B], F32)
            with tc.tile_pool(name="psD", bufs=1, space="PSUM") as psD:
                rbp = psD.tile([128, HL * B], F32, tag="rbp")
                nc.tensor.matmul(rbp[:], onesf[:], rec[:], start=True, stop=True)
                nc.vector.tensor_tensor(att[:], vT[:], ebs[:], MUL)
                nc.vector.tensor_tensor(att[:], att[:], atsb[:], ADD)
                nc.vector.tensor_tensor(att[:], att[:], rbp[:], MUL)
            nc.vector.tensor_copy(att_bf[:], att[:])

            with tc.tile_pool(name="psC", bufs=3, space="PSUM") as psC:
                for n in range(8):
                    opsn = psC.tile([B, 512], F32, tag="ops")
                    for h in range(HL):
                        nc.tensor.matmul(
                            opsn[:],
                            att_bf[:, h * B : (h + 1) * B],
                            wo_tiles[h][:, n * 512 : (n + 1) * 512],
                            start=(h == 0),
                            stop=(h == HL - 1),
                        )
                    outc = smp.tile([B, 512], F32, tag="outc")
                    if n % 2:
                        nc.scalar.copy(outc[:], opsn[:])
                    else:
                        nc.vector.tensor_copy(outc[:], opsn[:])
                    nc.sync.dma_start(
                        out=out_part[:, n * 512 : (n + 1) * 512], in_=outc[:]
                    )

    _split_excess_waits(nc)
    return nc


def _host_prep(hidden, W_pack, o_proj_weight, k_cache, v_cache, hist, block_offsets):
    """Build the 8 per-core input maps (numpy only)."""
    hidden = np.asarray(hidden, np.float32)
    W_pack = np.asarray(W_pack, np.float32)
    o_proj_weight = np.asarray(o_proj_weight, np.float32)
    k_cache = np.asarray(k_cache, np.float32)
    v_cache = np.asarray(v_cache, np.float32)
    hist = np.asarray(hist, np.int64)
    block_offsets = np.asarray(block_offsets, np.int64)

    # Process requests in descending-pairs order: the device sees requests
    # pre-permuted (hidden rows, rope tables, mask, kv packing), so the
    # attention tail runs on the smallest requests; host unpermutes output.
    pairs0 = np.array([int(h + 127) // 128 for h in hist])
    perm = np.argsort(-pairs0, kind="stable")
    hidden = hidden[perm]
    hist = hist[perm]
    block_offsets = block_offsets[perm]
    pairs = [int(p) for p in pairs0[perm]]
    SP = sum(pairs)

    # rope tables, scale folded into the q tables
    inv_freq = 1.0 / (ROPE_BASE ** (np.arange(0, D, 2, dtype=np.float32) / D))
    ang = hist.astype(np.float32)[:, None] * inv_freq[None, :]        # [B, 64]
    cos128 = np.concatenate([np.cos(ang), np.cos(ang)], -1)           # [B, 128]
    sin128 = np.concatenate([np.sin(ang), np.sin(ang)], -1)
    sign = np.concatenate([-np.ones(64), np.ones(64)]).astype(np.float32)
    sc = 1.0 / math.sqrt(D)
    tile_h = lambda x: np.tile(x, (1, HL)).astype(np.float32)         # [B, 512]
    cs = np.concatenate(
        [tile_h(cos128 * sc), tile_h(sin128 * sign * sc),
         tile_h(cos128), tile_h(sin128 * sign)], -1,
    )                                                                 # [B, 2048]

    # count of invalid (host-zeroed) kv slots per request: each contributes
    # exp(0)=1 to the device's denominator sum
    ninv = (np.array(pairs) * 128 - hist).astype(np.float32)          # [B]
    ninv = np.tile(ninv, HL)[None, :]                                 # [1, HL*B]

    hT = np.ascontiguousarray(hidden.T).astype(BF_NP)                 # [4096, 32]
    hT = np.ascontiguousarray(hT.reshape(KT, 128, B).transpose(1, 0, 2))

    # gather caches via the block table (b-major), slice heads per core
    k_all = k_cache[block_offsets.reshape(-1)]                        # [512,64,32,128]
    v_all = v_cache[block_offsets.reshape(-1)]

    ident = np.eye(B, dtype=np.float32)

    in_maps = []
    for c in range(NCORES):
        h0 = c * HL
        qcols = np.arange(h0 * D, (h0 + HL) * D)
        wp_c = np.concatenate(
            [W_pack[:, qcols], W_pack[:, HID + qcols], W_pack[:, 2 * HID + qcols]],
            axis=1,
        ).astype(BF_NP)                                               # [4096, 1536]
        wp_c = np.ascontiguousarray(
            wp_c.reshape(KT, 128, 3 * HL * D).transpose(1, 0, 2)
        )                                                             # [128,KT,1536]

        wo_c = np.ascontiguousarray(o_proj_weight[:, qcols].T).astype(BF_NP)
        wo_c = wo_c.reshape(HL, 128, HID)                             # [4,128,4096]

        kc = k_all[:, :, h0 : h0 + HL, :]                             # [512,64,4,128]
        vc = v_all[:, :, h0 : h0 + HL, :]
        kc = kc.reshape(B, PAIRS * 128, HL, D).copy()                 # pair-major
        vc = vc.reshape(B, PAIRS * 128, HL, D).copy()
        # zero invalid positions: they score exp(0)=1 against V=0, accounted
        # for by the ninv denominator correction
        pos = np.arange(PAIRS * 128)
        inval = pos[None, :] >= hist[:, None]                         # [B, S]
        kc[inval] = 0.0
        vc[inval] = 0.0
        kc = kc.reshape(B, PAIRS, 128, HL, D)
        vc = vc.reshape(B, PAIRS, 128, HL, D)
        # kv packed: [128, 2, SP, HL, 128] bf16; K part is [d, pair, h, s],
        # V part is [s, pair, h, d]
        kv_c = np.empty((128, 2, max(SP, 1), HL, 128), BF_NP)
        for b in range(B):
            pb = pairs[b]
            if pb == 0:
                continue
            o = sum(pairs[:b])
            kb = kc[b, :pb].astype(BF_NP)                             # [pb,128,4,128]
            vb = vc[b, :pb].astype(BF_NP)
            kv_c[:, 0, o : o + pb] = kb.transpose(3, 0, 2, 1)         # d,pair,h,s
            kv_c[:, 1, o : o + pb] = vb.transpose(1, 0, 2, 3)         # s,pair,h,d
        in_maps.append({
            "hT": hT, "wp": wp_c, "wo": wo_c, "kv": kv_c,
            "cs": cs, "ninv": ninv, "ident": ident,
        })
    return pairs, perm, in_maps


def kernel(hidden_states, W_pack, o_proj_weight, k_cache, v_cache,
           history_lengths, block_offsets):
    global LAST_RESULTS
    pairs, perm, in_maps = _host_prep(
        hidden_states, W_pack, o_proj_weight, k_cache, v_cache,
        history_lengths, block_offsets,
    )
    nc = _build_nc(pairs)
    trace = bool(int(os.environ.get("KERNEL_TRACE", "0")))
    res = run_bass_kernel_spmd(nc, in_maps, list(range(NCORES)), trace=trace)
    LAST_RESULTS = res
    outp = np.zeros((B, HID), np.float32)
    for c in range(NCORES):
        outp += res.results[c]["out_part"]
    out = np.zeros((B, HID), np.float32)
    out[perm] = outp                      # undo the descending-pairs permute
    return out


# revision 50
# speedup vs baseline: 1.1144x; 1.1144x over previous
"""Paged decode attention (nn_Attention_5626407157951) on 8 Trainium2 cores.

Tensor-parallel over heads: each core owns 4 of 32 heads. All operands are
bf16 single-term matmuls (no hi/lo error compensation) — rel err ~5e-3,
well inside the 2e-2 gate — which cuts both HBM bytes (81->53 MB/core)
and PE instruction count ~3x vs the compensated version.

Per core:
  qkv = hidden @ W_pack[:, own cols]       (bf16, fp32 acc)
  rotary(q, k) at pos=hist                 (DVE, fp32; host-built cos/sin)
  scores_T[s, (h,pair)] = K_cache^T q      (PE, K stationary bf16, q moving)
  softmax without max-subtraction; the last pair's validity mask is applied
  as the ACT bias operand of the exp; new token handled analytically:
      out = (sum_s exp(s)*v_s + e_new*v_new) / (sum_s exp(s) + e_new)
  out_partial = attn @ o_proj[:, own dims].T ; host sums the 8 partials.

DMA queue split: KV cache streams on the Sync HWDGE ring; weights
(W_pack, o_proj, hT) + outputs ride the Scalar HWDGE ring so the two
FIFO streams overlap and weight loads never queue behind KV prefetch.
KV pairs are packed compactly (only valid pairs shipped), K as
[d, pair, h, s] and V as [s, pair, h, d] so each per-request DMA is one
contiguous run per partition.
"""

import math
import os

import ml_dtypes
import numpy as np

import concourse.bass as bass
import concourse.mybir as mybir
import concourse.tile as tile
from concourse.bass_utils import run_bass_kernel_spmd
from concourse.vector_clock import ScopedClock

B = 32          # batch (decode requests)
H = 32          # total heads
HL = 4          # heads per core
D = 128         # head dim
HID = 4096
BS = 64         # cache block size
NBLK = 16       # blocks per request
NCORES = 8
KT = HID // 128         # 32 contraction tiles for qkv proj
PAIRS = NBLK // 2       # 8 block-pairs (128 tokens each) per request
ROPE_BASE = 10000.0
KV_PRE_ISSUE = 2        # kv groups queued before phase 1 (keep W_pack fast)
KV_DEPTH = 6            # kv group tiles in flight during attention
WP_CHUNK = 4            # kt tiles per W_pack DMA
GROUP_PAIRS = 6         # pack per-request kv DMAs to >= this many pairs
GROUP_CAP = 8           # ...but never beyond this many pairs per group
PIPE = 2                # attention software-pipeline depth (PV lags scores)

F32 = mybir.dt.float32
BF = mybir.dt.bfloat16
BF_NP = ml_dtypes.bfloat16
EXP_FN = mybir.ActivationFunctionType.Exp
MUL = mybir.AluOpType.mult
ADD = mybir.AluOpType.add
SUB = mybir.AluOpType.subtract

LAST_RESULTS = None  # test harness peeks at this for profiling info

# ---------------------------------------------------------------------------
# This walrus build accepts very few sync-waits per instruction; the Tile
# kernel-tail drain accumulates one wait per sem lane. Split the waits over
# several drain instructions (all before the barrier, so semantics hold).
_MAX_DRAIN_WAITS = 1


def _patched_drain_and_barrier(self, tick_clock, wait_clock):
    nc = self.nc
    drain_inst = nc.sync.drain()
    wait_clock.add_sem_waits(
        drain_inst.ins, ScopedClock({None: tick_clock.global_clock})
    )
    si = drain_inst.ins.sync_info
    if si is not None and si.on_wait and len(si.on_wait) > _MAX_DRAIN_WAITS:
        waits = list(si.on_wait)
        drain_inst.ins.sync_info = mybir.SyncInfo(
            on_wait=waits[:_MAX_DRAIN_WAITS], on_update=list(si.on_update or [])
        )
        rest = waits[_MAX_DRAIN_WAITS:]
        for i in range(0, len(rest), _MAX_DRAIN_WAITS):
            extra = nc.sync.drain()
            extra.ins.sync_info = mybir.SyncInfo(
                on_wait=rest[i : i + _MAX_DRAIN_WAITS], on_update=[]
            )
    nc.all_engine_barrier()
    popped = nc._tile_sem_poison_stack.pop()
    assert popped is self._sem_poison
    nc.clear_and_free_semaphores(list(self.sems.allocated().values()))
    nc.all_engine_barrier()


tile.TileContext._drain_and_barrier = _patched_drain_and_barrier


def _split_excess_waits(nc, limit=1):
    """Walrus rejects instructions carrying more than ~1 sync wait. Hoist the
    excess onto NoOps inserted just before, on the same engine queue (the
    queue blocks on them first, so semantics are identical)."""
    for fn in nc.m.functions:
        for bb in fn.blocks:
            out = []
            changed = False
            for inst in list(bb.instructions):
                si = getattr(inst, "sync_info", None)
                if si is not None and si.on_wait and len(si.on_wait) > limit:
                    waits = list(si.on_wait)
                    extra, keep = waits[:-limit], waits[-limit:]
                    for i in range(0, len(extra), limit):
                        nop = mybir.InstNoOp(
                            name=nc.get_next_instruction_name(),
                            ins=[], outs=[], engine=inst.engine,
                            sync_info=mybir.SyncInfo(
                                on_wait=extra[i : i + limit], on_update=[]
                            ),
                        )
                        nc.register_instruction(nop)
                        out.append(nop)
                    inst.sync_info = mybir.SyncInfo(
                        on_wait=keep, on_update=list(si.on_update or [])
                    )
                    changed = True
                out.append(inst)
            if changed:
                bb.instructions = out
# ---------------------------------------------------------------------------


def _build_nc(pairs):
    """Build the SPMD bass module. `pairs[b]` = number of 128-token cached
    pairs for request b (same on every core; head split is via input data)."""
    nc = bass.Bass()

    HD = HL * D  # 512 local attention dims
    SP = sum(pairs)
    offs = np.concatenate([[0], np.cumsum(pairs)]).astype(int)

    def param(name, shape, dt):
        return nc.declare_dram_parameter(name, list(shape), dt, isOutput=False)

    hT = param("hT", [128, KT, B], BF)
    wp = param("wp", [128, KT, 3 * HD], BF)
    wo = param("wo", [HL, 128, HID], BF)
    kvp = param("kv", [128, 2, max(SP, 1), HL, 128], BF)
    cs = param("cs", [B, 4 * HD], F32)
    ninvp = param("ninv", [1, HL * B], F32)
    identp = param("ident", [B, B], F32)
    out_part = nc.declare_dram_parameter("out_part", [B, HID], F32, isOutput=True)

    # Pack per-request kv loads into >= GROUP_PAIRS-pair DMA groups so small
    # requests don't pay per-transfer fixed latency each.
    groups = []          # list of lists of b
    cur, cur_p = [], 0
    for b in range(B):
        if pairs[b] == 0:
            continue
        if cur and cur_p + pairs[b] > GROUP_CAP:
            groups.append(cur)
            cur, cur_p = [], 0
        cur.append(b)
        cur_p += pairs[b]
        if cur_p >= GROUP_PAIRS:
            groups.append(cur)
            cur, cur_p = [], 0
    if cur:
        groups.append(cur)
    binfo = {}           # b -> (group idx, local pair offset)
    for gi, bs in enumerate(groups):
        lo = 0
        for b in bs:
            binfo[b] = (gi, lo)
            lo += pairs[b]
    gpairs = [sum(pairs[b] for b in bs) for bs in groups]
    goffs = [offs[bs[0]] for bs in groups]

    with tile.TileContext(nc) as tc:
        with (
            tc.tile_pool(name="const", bufs=1) as cpool,
            tc.tile_pool(name="work", bufs=1) as wpool,
            tc.tile_pool(name="wop", bufs=4) as wop,
            tc.tile_pool(name="kvpre", bufs=KV_PRE_ISSUE) as kvpre,
            tc.tile_pool(name="small", bufs=4) as smp,
        ):
            # ---- constants (sync ring, ahead of the KV stream) ----
            ident = cpool.tile([B, B], F32)
            nc.sync.dma_start(out=ident[:], in_=identp[:])
            ninv_sb = cpool.tile([1, HL * B], F32)
            nc.sync.dma_start(out=ninv_sb[:], in_=ninvp[:])
            cs_sb = cpool.tile([B, 4 * HD], F32)
            nc.sync.dma_start(out=cs_sb[:], in_=cs[:])
            # scalar ring leads with what the PE needs first
            hT_sb = cpool.tile([128, KT, B], BF)
            nc.scalar.dma_start(out=hT_sb[:], in_=hT[:])
            ones = cpool.tile([128, 1], BF)
            nc.vector.memset(ones[:], 1.0)
            onesf = cpool.tile([1, HL * B], F32)
            nc.vector.memset(onesf[:], 1.0)

            # KV loads: one DMA per group, K+V together, sync ring. The first
            # couple go through a small dedicated pool ahead of phase 1; the
            # main (deep) pool opens after W_pack staging closes, reusing its
            # SBUF arena.
            g_tiles = {}
            kvmain = [None]

            def load_g(gi):
                gp = gpairs[gi]
                o = goffs[gi]
                pool = kvmain[0] if kvmain[0] is not None else kvpre
                t = pool.tile([128, 2, gp, HL, 128], BF, tag="kv")
                nc.sync.dma_start(out=t[:], in_=kvp[:, :, o : o + gp, :, :])
                g_tiles[gi] = t

            for gi in range(min(KV_PRE_ISSUE, len(groups))):
                load_g(gi)

            # o_proj weights ride the sync ring right behind the first kv
            # groups: they stream during phase 1 and never block the scalar
            # queue (which phase 2's rope copies need).
            wo_tiles = {}
            for i in range(HL):
                t = wop.tile([128, HID], BF, tag="wot")
                nc.sync.dma_start(out=t[:], in_=wo[i])
                wo_tiles[i] = t

            # accumulators written per-b, read in the epilogue
            atsb = wpool.tile([128, HL * B], F32)   # cached attn, col h*32+b
            nc.vector.memset(atsb[:], 0.0)
            dnm = wpool.tile([1, HL * B], F32)      # cached denom, col h*32+b
            nc.vector.memset(dnm[:], 0.0)

            with tc.tile_pool(name="psA", bufs=1, space="PSUM") as psA:
                # PE warmup transpose so `ident` is observed by PE before the
                # real (fp32, single-wait-slot) transposes below.
                tp0 = psA.tile([B, B], F32, tag="tp0")
                nc.tensor.transpose(tp0[:], ident[:], ident[:])

                # ---- phase 1: qkv = hidden @ W_pack (bf16, chunked DMA) ----
                qkv_ps = psA.tile([B, 3 * HD], F32, tag="qkv")
                NCH = KT // WP_CHUNK
                with tc.tile_pool(name="wtiles", bufs=3) as wtp:
                    for ch in range(NCH):
                        k0 = ch * WP_CHUNK
                        wpt = wtp.tile([128, WP_CHUNK, 3 * HD], BF, tag="wpt")
                        nc.scalar.dma_start(
                            out=wpt[:], in_=wp[:, k0 : k0 + WP_CHUNK, :]
                        )
                        for j in range(WP_CHUNK):
                            for n in range(3):
                                nc.tensor.matmul(
                                    qkv_ps[:, n * HD : (n + 1) * HD],
                                    hT_sb[:, k0 + j, :],
                                    wpt[:, j, n * HD : (n + 1) * HD],
                                    start=(k0 + j == 0),
                                    stop=(k0 + j == KT - 1),
                                )
                qkv_sb = wpool.tile([B, 3 * HD], F32)
                nc.vector.tensor_copy(qkv_sb[:], qkv_ps[:])

            # phase 1 psum + W_pack staging closed; open the deep kv pool
            # (reuses the staging arena) for phase 2 onward. Entered manually
            # so it spans phase 2 + 3; closed after the attention loop.
            kvm_cm = tc.tile_pool(name="kvm", bufs=KV_DEPTH - KV_PRE_ISSUE + 1)
            kvmain[0] = kvm_cm.__enter__()

            if True:
                # ---- phase 2: rotary (fp32, DVE) + transposes ----
                def rope(src_off, cs_off):
                    src = qkv_sb[:, src_off : src_off + HD]
                    t1 = wpool.tile([B, HD], F32, tag="rope_t1")
                    nc.vector.tensor_tensor(
                        t1[:], src, cs_sb[:, cs_off : cs_off + HD], MUL
                    )
                    sh = wpool.tile([B, HD], F32, tag="rope_sh")
                    sh4 = sh[:].rearrange("b (h d) -> b h d", h=HL)
                    sr4 = qkv_sb[:, src_off : src_off + HD].rearrange(
                        "b (h d) -> b h d", h=HL
                    )
                    # rotate-half copies on the (idle) scalar engine so they
                    # overlap the DVE multiplies
                    nc.scalar.copy(sh4[:, :, 0:64], sr4[:, :, 64:128])
                    nc.scalar.copy(sh4[:, :, 64:128], sr4[:, :, 0:64])
                    nc.vector.tensor_tensor(
                        sh[:], sh[:], cs_sb[:, cs_off + HD : cs_off + 2 * HD], MUL
                    )
                    nc.vector.tensor_tensor(
                        qkv_sb[:, src_off : src_off + HD], t1[:], sh[:], ADD
                    )

                rope(0, 0)          # q (scale folded into tables)
                rope(HD, 2 * HD)    # k

                # PE transposes -> [128(d), (h,b)] fp32 tiles (pipelined)
                qT = wpool.tile([128, HL * B], F32)
                kT = wpool.tile([128, HL * B], F32)
                vT = wpool.tile([128, HL * B], F32)
                with tc.tile_pool(name="psT", bufs=2, space="PSUM") as psT:
                    for off, dst in ((0, qT), (HD, kT), (2 * HD, vT)):
                        for h in range(HL):
                            tp = psT.tile([128, B], F32, tag="tp")
                            inp = qkv_sb[:, off + h * D : off + (h + 1) * D]
                            nc.tensor.transpose(tp[:], inp, ident[:])
                            nc.vector.tensor_copy(
                                dst[:, h * B : (h + 1) * B], tp[:]
                            )

                qT_bf = wpool.tile([128, HL * B], BF)
                nc.vector.tensor_copy(qT_bf[:], qT[:])

                # new-token scores: e_new[(h,b)] = exp(q . k_new)
                prod = wpool.tile([128, HL * B], F32)
                nc.vector.tensor_tensor(prod[:], qT[:], kT[:], MUL)
                prod_bf = wpool.tile([128, HL * B], BF)
                nc.vector.tensor_copy(prod_bf[:], prod[:])
                with tc.tile_pool(name="psA2", bufs=1, space="PSUM") as psA2:
                    sn_ps = psA2.tile([1, HL * B], F32, tag="sn")
                    nc.tensor.matmul(
                        sn_ps[:], ones[:], prod_bf[:], start=True, stop=True
                    )
                    e_new = wpool.tile([1, HL * B], F32)
                    nc.scalar.activation(e_new[:], sn_ps[:], EXP_FN)
                    # broadcast e_new across partitions now (sbuf copy)
                    ebp = psA2.tile([128, HL * B], F32, tag="ebp")
                    nc.tensor.matmul(
                        ebp[:], onesf[:], e_new[:], start=True, stop=True
                    )
                    ebs = wpool.tile([128, HL * B], F32)
                    nc.vector.tensor_copy(ebs[:], ebp[:])

            dtot = wpool.tile([1, HL * B], F32)
            rec = wpool.tile([1, HL * B], F32)
            att_bf = wpool.tile([128, HL * B], BF)

            # ---- phase 3: group-batched paged attention ----
            # One scores->mask->exp->PV round trip per kv DMA group (not per
            # request), software-pipelined PIPE groups deep so the PE never
            # waits on an exp it just issued.
            with (
                tc.tile_pool(name="psS", bufs=3, space="PSUM") as psS,
                tc.tile_pool(name="psV", bufs=3, space="PSUM") as psV,
                tc.tile_pool(name="psB2", bufs=2, space="PSUM") as psB2,
            ):
                def do_pv(ent):
                    gi, bs, kvt, ph = ent
                    gp = gpairs[gi]
                    nb = len(bs)
                    b0 = bs[0]
                    atp = psV.tile([128, HL, nb], F32, tag="atp")
                    for h in range(HL):
                        for j, b in enumerate(bs):
                            lo = offs[b] - goffs[gi]
                            pb = pairs[b]
                            for p in range(pb):
                                nc.tensor.matmul(
                                    atp[:, h, j : j + 1],
                                    kvt[:, 1, lo + p, h, :],
                                    ph[:, h, lo + p : lo + p + 1],
                                    start=(p == 0), stop=(p == pb - 1),
                                )
                    nc.vector.tensor_copy(
                        atsb[:].rearrange("d (h b2) -> d h b2", h=HL)
                        [:, :, b0 : b0 + nb],
                        atp[:],
                    )
                    # denominators: column sums of probs, then per-b segments
                    dsp = psB2.tile([1, HL * gp], F32, tag="dsp")
                    nc.tensor.matmul(
                        dsp[:], ones[:], ph[:].rearrange("s h p -> s (h p)"),
                        start=True, stop=True,
                    )
                    dspv = dsp[:].rearrange("o (h p) -> o h p", h=HL)
                    for b in bs:
                        lo = offs[b] - goffs[gi]
                        nc.vector.reduce_sum(
                            dnm[:].rearrange("o (h b2) -> o h b2", h=HL)[:, :, b],
                            dspv[:, :, lo : lo + pairs[b]],
                            axis=mybir.AxisListType.X,
                        )

                stage = []
                for gi, bs in enumerate(groups):
                    if gi not in g_tiles:
                        load_g(gi)
                    for gn in range(gi + 1, min(gi + KV_DEPTH - 1, len(groups))):
                        if gn not in g_tiles:
                            load_g(gn)
                    kvt = g_tiles[gi]
                    gp = gpairs[gi]
                    o = goffs[gi]

                    # scores^T: [128(s), (h, pair)] for the whole group
                    scp = psS.tile([128, HL, gp], F32, tag="scp")
                    for h in range(HL):
                        for b in bs:
                            lo = offs[b] - o
                            qh = qT_bf[:, h * B + b : h * B + b + 1]
                            for p in range(pairs[b]):
                                nc.tensor.matmul(
                                    scp[:, h, lo + p : lo + p + 1],
                                    kvt[:, 0, lo + p, h, :],
                                    qh, start=True, stop=True,
                                )

                    # exp -> probs (bf16), straight from PSUM. No mask: the
                    # host zeroed K/V at invalid positions, so they land as
                    # exp(0)=1 times V=0; the denominator over-count is a
                    # host-known constant removed in the epilogue.
                    ph = smp.tile([128, HL, gp], BF, tag="ph")
                    nc.scalar.activation(ph[:], scp[:], EXP_FN)

                    stage.append((gi, bs, kvt, ph))
                    if len(stage) > PIPE:
                        do_pv(stage.pop(0))
                while stage:
                    do_pv(stage.pop(0))

            kvm_cm.__exit__(None, None, None)

            # ---- epilogue: add new token, normalize, project ----
            # dnm counts exp(0)=1 for each host-zeroed invalid slot; subtract
            # the known count, then add the new token's weight.
            nc.vector.tensor_tensor(dtot[:], dnm[:], ninv_sb[:], SUB)
            nc.vector.tensor_tensor(dtot[:], dtot[:], e_new[:], ADD)
            nc.vector.reciprocal(rec[:], dtot[:])
            att = wpool.tile([128, HL * B], F32)
            with tc.tile_pool(name="psD", bufs=1, space="PSUM") as psD:
                rbp = psD.tile([128, HL * B], F32, tag="rbp")
                nc.tensor.matmul(rbp[:], onesf[:], rec[:], start=True, stop=True)
                nc.vector.tensor_tensor(att[:], vT[:], ebs[:], MUL)
                nc.vector.tensor_tensor(att[:], att[:], atsb[:], ADD)
                nc.vector.tensor_tensor(att[:], att[:], rbp[:], MUL)
            nc.vector.tensor_copy(att_bf[:], att[:])

            with tc.tile_pool(name="psC", bufs=3, space="PSUM") as psC:
                for n in range(8):
                    opsn = psC.tile([B, 512], F32, tag="ops")
                    for h in range(HL):
                        nc.tensor.matmul(
                            opsn[:],
                            att_bf[:, h * B : (h + 1) * B],
                            wo_tiles[h][:, n * 512 : (n + 1) * 512],
                            start=(h == 0),
                            stop=(h == HL - 1),
                        )
                    outc = smp.tile([B, 512], F32, tag="outc")
                    if n % 2:
                        nc.scalar.copy(outc[:], opsn[:])
                    else:
                        nc.vector.tensor_copy(outc[:], opsn[:])
                    nc.sync.dma_start(
                        out=out_part[:, n * 512 : (n + 1) * 512], in_=outc[:]
                    )

    _split_excess_waits(nc)
    return nc


def _host_prep(hidden, W_pack, o_proj_weight, k_cache, v_cache, hist, block_offsets):
    """Build the 8 per-core input maps (numpy only)."""
    hidden = np.asarray(hidden, np.float32)
    W_pack = np.asarray(W_pack, np.float32)
    o_proj_weight = np.asarray(o_proj_weight, np.float32)
    k_cache = np.asarray(k_cache, np.float32)
    v_cache = np.asarray(v_cache, np.float32)
    hist = np.asarray(hist, np.int64)
    block_offsets = np.asarray(block_offsets, np.int64)

    # Process requests in descending-pairs order: the device sees requests
    # pre-permuted (hidden rows, rope tables, mask, kv packing), so the
    # attention tail runs on the smallest requests; host unpermutes output.
    pairs0 = np.array([int(h + 127) // 128 for h in hist])
    perm = np.argsort(-pairs0, kind="stable")
    hidden = hidden[perm]
    hist = hist[perm]
    block_offsets = block_offsets[perm]
    pairs = [int(p) for p in pairs0[perm]]
    SP = sum(pairs)

    # rope tables, scale folded into the q tables
    inv_freq = 1.0 / (ROPE_BASE ** (np.arange(0, D, 2, dtype=np.float32) / D))
    ang = hist.astype(np.float32)[:, None] * inv_freq[None, :]        # [B, 64]
    cos128 = np.concatenate([np.cos(ang), np.cos(ang)], -1)           # [B, 128]
    sin128 = np.concatenate([np.sin(ang), np.sin(ang)], -1)
    sign = np.concatenate([-np.ones(64), np.ones(64)]).astype(np.float32)
    sc = 1.0 / math.sqrt(D)
    tile_h = lambda x: np.tile(x, (1, HL)).astype(np.float32)         # [B, 512]
    cs = np.concatenate(
        [tile_h(cos128 * sc), tile_h(sin128 * sign * sc),
         tile_h(cos128), tile_h(sin128 * sign)], -1,
    )                                                                 # [B, 2048]

    # count of invalid (host-zeroed) kv slots per request: each contributes
    # exp(0)=1 to the device's denominator sum
    ninv = (np.array(pairs) * 128 - hist).astype(np.float32)          # [B]
    ninv = np.tile(ninv, HL)[None, :]                                 # [1, HL*B]

    hT = np.ascontiguousarray(hidden.T).astype(BF_NP)                 # [4096, 32]
    hT = np.ascontiguousarray(hT.reshape(KT, 128, B).transpose(1, 0, 2))

    # gather caches via the block table (b-major), slice heads per core
    k_all = k_cache[block_offsets.reshape(-1)]                        # [512,64,32,128]
    v_all = v_cache[block_offsets.reshape(-1)]

    ident = np.eye(B, dtype=np.float32)

    in_maps = []
    for c in range(NCORES):
        h0 = c * HL
        qcols = np.arange(h0 * D, (h0 + HL) * D)
        wp_c = np.concatenate(
            [W_pack[:, qcols], W_pack[:, HID + qcols], W_pack[:, 2 * HID + qcols]],
            axis=1,
        ).astype(BF_NP)                                               # [4096, 1536]
        wp_c = np.ascontiguousarray(
            wp_c.reshape(KT, 128, 3 * HL * D).transpose(1, 0, 2)
        )                                                             # [128,KT,1536]

        wo_c = np.ascontiguousarray(o_proj_weight[:, qcols].T).astype(BF_NP)
        wo_c = wo_c.reshape(HL, 128, HID)                             # [4,128,4096]

        kc = k_all[:, :, h0 : h0 + HL, :]                             # [512,64,4,128]
        vc = v_all[:, :, h0 : h0 + HL, :]
        kc = kc.reshape(B, PAIRS * 128, HL, D).copy()                 # pair-major
        vc = vc.reshape(B, PAIRS * 128, HL, D).copy()
        # zero invalid positions: they score exp(0)=1 against V=0, accounted
        # for by the ninv denominator correction
        pos = np.arange(PAIRS * 128)
        inval = pos[None, :] >= hist[:, None]                         # [B, S]
        kc[inval] = 0.0
        vc[inval] = 0.0
        kc = kc.reshape(B, PAIRS, 128, HL, D)
        vc = vc.reshape(B, PAIRS, 128, HL, D)
        # kv packed: [128, 2, SP, HL, 128] bf16; K part is [d, pair, h, s],
        # V part is [s, pair, h, d]
        kv_c = np.empty((128, 2, max(SP, 1), HL, 128), BF_NP)
        for b in range(B):
            pb = pairs[b]
            if pb == 0:
                continue
            o = sum(pairs[:b])
            kb = kc[b, :pb].astype(BF_NP)                             # [pb,128,4,128]
            vb = vc[b, :pb].astype(BF_NP)
            kv_c[:, 0, o : o + pb] = kb.transpose(3, 0, 2, 1)         # d,pair,h,s
            kv_c[:, 1, o : o + pb] = vb.transpose(1, 0, 2, 3)         # s,pair,h,d
        in_maps.append({
            "hT": hT, "wp": wp_c, "wo": wo_c, "kv": kv_c,
            "cs": cs, "ninv": ninv, "ident": ident,
        })
    return pairs, perm, in_maps


def kernel(hidden_states, W_pack, o_proj_weight, k_cache, v_cache,
           history_lengths, block_offsets):
    global LAST_RESULTS
    pairs, perm, in_maps = _host_prep(
        hidden_states, W_pack, o_proj_weight, k_cache, v_cache,
        history_lengths, block_offsets,
    )
    nc = _build_nc(pairs)
    trace = bool(int(os.environ.get("KERNEL_TRACE", "0")))
    res = run_bass_kernel_spmd(nc, in_maps, list(range(NCORES)), trace=trace)
    LAST_RESULTS = res
    outp = np.zeros((B, HID), np.float32)
    for c in range(NCORES):
        outp += res.results[c]["out_part"]
    out = np.zeros((B, HID), np.float32)
    out[perm] = outp                      # undo the descending-pairs permute
    return out
